# revision 18
# baseline (speedup 1.0000x reference)
"""Submanifold 3x3x3 sparse conv (gnn_message_passing) + BatchNorm + LeakyReLU
on 8 Trainium2 NeuronCores — pair-list + DMA scatter-add formulation.

Per core (~25088 rows, ~61k valid non-self pairs):
  * Whole connected components partitioned across 8 cores (LPT by pair count);
    within a core, components packed into 7 regions of 3584 rows (gather
    locality); pair cells keyed (super-region in {0,1}, k) so only 52
    scatter calls are needed.
  * Cell budgets = max over the 8 cores (shared program), padded to x128;
    pad slots gather the zero row and scatter onto a junk group.
  * dma_gather streams bf16 feature rows; stage 1 does per-k matmuls
    W[k]^T @ feat[src] -> P2 [c, pair] in PSUM; PE-transposes 128-pair
    chunks -> p2t [pair, c] per-cell tiles.
  * dma_scatter_add (SBUF-dst, tokens_per_rank=128, parity-split) adds each
    pair's contribution onto accumulator rows (partition dst%128, parity
    (dst//128)%2, free group dst//256).  One call per cell: within a cell
    every dst is unique (a voxel has at most one neighbor per offset), so
    the CCE RMW descriptors never collide.  Calls ping-pong between two
    accumulator pairs A/B so consecutive calls pipeline instead of
    serializing on the DMA completion semaphore.
  * Self contribution (k=13) initializes A via flipped matmuls
    (stationary = pretransposed feature chunk, moving = W[13]).
  * Tail: A += B merges, per-channel stats via strided [p, c, g]
    tensor_reduce + ACT squares, ones-matmul cross-partition finalize,
    all-reduce, then BN affine + LeakyReLU with channel-broadcast tiles;
    output written as [dst, c] f32 rows.
"""

import os
import numpy as np
import ml_dtypes

ABLATE = set(os.environ.get("KABLATE", "").split(","))
C = 128
K = 27
EPS = 1e-4
LEAK = 0.333
N_CORES = 8
SELF_K = 13
N_REG = 7
REG_ROWS = 3584                 # 28 chunks of 128
SHARD = N_REG * REG_ROWS        # 25088 rows per core
TABLE_ROWS = SHARD + 128
ZERO_ROW = TABLE_ROWS - 1
JUNK_DST = SHARD                # scatter junk slot: partition 0, group 98
N_GRP = SHARD // 256            # 98 real groups
GROUPS = N_GRP + 1              # + junk group
N_TOTAL = 200_000
KG_LIST = [k for k in range(K) if k != SELF_K]
KG = len(KG_LIST)
N_SREG = 2
SREG_OF_REG = np.array([0, 0, 0, 0, 1, 1, 1])
SREG_GRP = [(0, 56), (56, 98)]  # group ranges per super-region
SCAT_MAX = 1536
GSLAB = 4096                    # gather slab (8 x 512)
OPT = {"queues": 4, "gath_bufs": 4, "p2t_bufs": 6}


class Plan:
    def __init__(self, budgets):
        # budgets: [N_SREG, KG] slot counts, multiples of 128
        assert (budgets % 128 == 0).all()
        self.budgets = budgets
        self.sreg_pad = budgets.sum(axis=1)
        self.sreg_off = np.concatenate([[0], np.cumsum(self.sreg_pad)])[:-1]
        self.p_total = int(self.sreg_pad.sum())
        self.cell_off = np.zeros((N_SREG, KG), np.int64)
        self.cell_off[:, 1:] = np.cumsum(budgets, axis=1)[:, :-1]
        self.cmax = int(budgets.max())


def _partition_components(nb, n, n_cores, shard_cap):
    """Whole-component LPT partition by pair count. (members, labels), ok."""
    import scipy.sparse as sp
    import scipy.sparse.csgraph as csg
    import heapq

    valid = nb >= 0
    paircnt = valid[:, KG_LIST].sum(1).astype(np.int64)
    ii, kk = np.nonzero(valid)
    jj = nb[ii, kk]
    m = kk != SELF_K
    g = sp.coo_matrix((np.ones(m.sum(), np.int8), (ii[m], jj[m])), shape=(n, n))
    _, labels = csg.connected_components(g, directed=False)
    sizes = np.bincount(labels)
    cpairs = np.bincount(labels, weights=paircnt).astype(np.int64)
    if sizes.max() > shard_cap:
        return None, False
    order = np.argsort(cpairs)[::-1]
    heap = [(0, 0, c) for c in range(n_cores)]
    heapq.heapify(heap)
    loads = np.zeros(n_cores, np.int64)
    rows = np.zeros(n_cores, np.int64)
    assign = np.empty(len(sizes), np.int32)
    for comp in order:
        tried = []
        placed = False
        while heap:
            load, _, c = heapq.heappop(heap)
            if rows[c] + sizes[comp] <= shard_cap:
                assign[comp] = c
                loads[c] += int(cpairs[comp])
                rows[c] += int(sizes[comp])
                heapq.heappush(heap, (loads[c], int(rows[c]), c))
                placed = True
                break
            tried.append((load, int(rows[c]), c))
        for t in tried:
            heapq.heappush(heap, t)
        if not placed:
            return None, False
    shard_of = assign[labels]
    members = [np.nonzero(shard_of == c)[0] for c in range(n_cores)]
    return (members, labels), True


def _pack_regions(comp_mem, paircnt_mem):
    """Pack whole components into N_REG bins of <=REG_ROWS rows, balancing
    pair counts (components >REG_ROWS split).  Returns (bin, pos)."""
    import heapq
    r_n = len(paircnt_mem)
    order = np.argsort(comp_mem, kind="stable")
    sc = comp_mem[order]
    starts = np.nonzero(np.r_[True, sc[1:] != sc[:-1]])[0]
    ends = np.r_[starts[1:], r_n]
    chunks = []
    for a, b in zip(starts, ends):
        for o in range(a, b, REG_ROWS):
            idxs = order[o:min(o + REG_ROWS, b)]
            chunks.append((int(paircnt_mem[idxs].sum()), idxs))
    chunks.sort(key=lambda t: -t[0])
    heap = [(0, 0, c) for c in range(N_REG)]
    heapq.heapify(heap)
    fill = np.zeros(N_REG, np.int64)
    loads = np.zeros(N_REG, np.int64)
    s_of = np.empty(r_n, np.int64)
    pos_of = np.empty(r_n, np.int64)
    for pc, idxs in chunks:
        tried = []
        while True:
            load, _, c = heapq.heappop(heap)
            if fill[c] + len(idxs) <= REG_ROWS:
                s_of[idxs] = c
                pos_of[idxs] = fill[c] + np.arange(len(idxs))
                fill[c] += len(idxs)
                loads[c] += pc
                heapq.heappush(heap, (int(loads[c]), int(fill[c]), c))
                break
            tried.append((load, int(fill[c]), c))
            if not heap:
                raise RuntimeError("region packing failed")
        for t in tried:
            heapq.heappush(heap, t)
    return s_of, pos_of


def _build_pairs(nb, members_labels):
    members, labels = members_labels
    n = nb.shape[0]
    cores = []
    counts = np.zeros((len(members), N_SREG, KG), np.int64)
    valid_all = nb >= 0
    paircnt = valid_all[:, KG_LIST].sum(1).astype(np.int64)
    for c, mem in enumerate(members):
        s_of, pos_of = _pack_regions(labels[mem], paircnt[mem])
        rows = s_of * REG_ROWS + pos_of
        loc = np.full(n, ZERO_ROW, np.int64)
        loc[mem] = rows
        nbm = nb[mem][:, KG_LIST]
        vm = nbm >= 0
        src_rows = loc[np.where(vm, nbm, 0)]
        ii, kk = np.nonzero(vm)
        p_src = src_rows[ii, kk]
        p_sreg = SREG_OF_REG[s_of[ii]]
        p_dst = rows[ii]
        p_k = kk
        assert (p_src < SHARD).all()
        counts[c] = np.bincount(p_sreg * KG + p_k,
                                minlength=N_SREG * KG).reshape(N_SREG, KG)
        cores.append({"mem": mem, "rows": rows, "p_src": p_src,
                      "p_sreg": p_sreg, "p_dst": p_dst, "p_k": p_k})
    return cores, counts


def _make_plan(counts):
    budgets = counts.max(axis=0)
    budgets = ((budgets + 127) // 128) * 128
    return Plan(budgets)


def _build_core_arrays(core, plan, feats_bf):
    slot_base = (plan.sreg_off[core["p_sreg"]]
                 + plan.cell_off[core["p_sreg"], core["p_k"]])
    key = core["p_sreg"] * KG + core["p_k"]
    order = np.lexsort((core["p_src"], key))
    sk = key[order]
    grp_start = np.searchsorted(sk, np.arange(N_SREG * KG))
    within = np.empty(len(sk), np.int64)
    within[order] = np.arange(len(sk)) - grp_start[sk]
    slot = slot_base + within

    idx_flat = np.full(plan.p_total, ZERO_ROW, np.int16)
    idx_flat[slot] = core["p_src"].astype(np.int16)
    dl_flat = np.full(plan.p_total, JUNK_DST, np.int16)
    dl_flat[slot] = core["p_dst"].astype(np.int16)

    idx_arr = np.tile(np.ascontiguousarray(idx_flat.reshape(-1, 16).T), (8, 1))
    dl_arr = np.tile(np.ascontiguousarray(dl_flat.reshape(-1, 16).T), (8, 1))

    table = np.zeros((TABLE_ROWS, C), ml_dtypes.bfloat16)
    table[core["rows"]] = feats_bf[core["mem"]]
    table_t = np.ascontiguousarray(table[:SHARD].T)
    return {"table": table, "table_t": table_t, "idx": idx_arr, "dl": dl_arr}


def emit_kernel(tc, out_ap, ins, plan, n_cores=N_CORES):
    import concourse.mybir as mybir
    from concourse.masks import make_identity

    nc = tc.nc
    F32 = mybir.dt.float32
    BF16 = mybir.dt.bfloat16
    I16 = mybir.dt.int16
    AG = GROUPS * 128

    table, table_t, idx, dl, w = (ins["table"], ins["table_t"], ins["idx"],
                                  ins["dl"], ins["w"])
    gamma, beta = ins["gamma"], ins["beta"]

    with (
        tc.tile_pool(name="const", bufs=1) as constp,
        tc.tile_pool(name="tabt", bufs=2) as tabtp,
        tc.tile_pool(name="gath", bufs=OPT["gath_bufs"]) as gathp,
        tc.tile_pool(name="p2sb", bufs=4) as p2sbp,
        tc.tile_pool(name="p2t", bufs=OPT["p2t_bufs"]) as p2tp,
        tc.tile_pool(name="ph2", bufs=3) as ph2p,
        tc.tile_pool(name="psum1", bufs=2, space="PSUM") as psum1,
        tc.tile_pool(name="psumT", bufs=2, space="PSUM") as psumT,
        tc.tile_pool(name="psumS", bufs=1, space="PSUM") as psumS,
        tc.tile_pool(name="psumF", bufs=1, space="PSUM") as psumF,
        tc.tile_pool(name="dram", bufs=1, space="DRAM") as dramp,
    ):
        w_sb = constp.tile([128, K * C], BF16)
        nc.sync.dma_start(w_sb[:].rearrange("ci (k co) -> ci k co", k=K),
                          w.rearrange("k ci co -> ci k co"))
        idx_all = constp.tile([128, plan.p_total // 16], I16)
        nc.sync.dma_start(idx_all[:], idx)
        dl_all = constp.tile([128, plan.p_total // 16], I16)
        nc.sync.dma_start(dl_all[:], dl)
        gamma_sb = constp.tile([128, 1], F32)
        nc.sync.dma_start(gamma_sb[:], gamma[:, None])
        beta_sb = constp.tile([128, 1], F32)
        nc.sync.dma_start(beta_sb[:], beta[:, None])
        identity = constp.tile([128, 128], BF16)
        make_identity(nc, identity[:])
        ones_row = constp.tile([1, 128], BF16)
        nc.vector.memset(ones_row[:], 1.0)
        ones_col = constp.tile([128, 1], BF16)
        nc.vector.memset(ones_col[:], 1.0)

        # accumulators: 2 ping-pong pairs x 2 parities
        acc_a0 = constp.tile([128, AG], BF16)
        acc_a1 = constp.tile([128, AG], BF16)
        acc_b0 = constp.tile([128, AG], BF16)
        acc_b1 = constp.tile([128, AG], BF16)
        acc = [[acc_a0, acc_a1], [acc_b0, acc_b1]]
        # A real groups are fully initialized by the self pass; zero only junk
        nc.vector.memset(acc_a0[:, N_GRP * 128:], 0)
        nc.vector.memset(acc_a1[:, N_GRP * 128:], 0)
        nc.vector.memset(acc_b0[:], 0)
        nc.vector.memset(acc_b1[:], 0)

        # ---- self init: A[parity] = feat[dst] @ W13 ----
        for gh in range(N_REG * 2):
            h0 = gh * (REG_ROWS // 2)
            tt = tabtp.tile([128, REG_ROWS // 2], BF16)
            nc.sync.dma_start(tt[:], table_t[:, h0:h0 + REG_ROWS // 2])
            for q in range(0, REG_ROWS // 2, 256):
                pss = psumS.tile([128, 256], F32)
                for cc in range(2):
                    nc.tensor.matmul(
                        pss[:, cc * 128:(cc + 1) * 128],
                        tt[:, q + cc * 128:q + (cc + 1) * 128],
                        w_sb[:, SELF_K * C:(SELF_K + 1) * C],
                        start=True, stop=True)
                gc = (h0 + q) // 128                # even
                for pa in range(2):
                    nc.scalar.copy(
                        acc[0][pa][:, (gc // 2) * 128:(gc // 2 + 1) * 128],
                        pss[:, pa * 128:(pa + 1) * 128])

        # ---- pipeline: gather slabs -> stage1 -> transpose -> scatter ----
        cell_idx = [0]

        def emit_gather(sr, si):
            lo = int(plan.sreg_off[sr]) + si * GSLAB
            hi = min(int(plan.sreg_off[sr]) + int(plan.sreg_pad[sr]),
                     lo + GSLAB)
            gt = gathp.tile([128, 1, GSLAB], BF16)
            for o in range(lo, hi, SCAT_MAX):
                e = min(hi, o + SCAT_MAX)
                if "gather" in ABLATE:
                    continue
                nc.gpsimd.dma_gather(
                    gt[:, :, o - lo:e - lo], table,
                    idx_all[:, o // 16:e // 16],
                    e - o, e - o, C,
                    transpose=True, single_packet=False, queue_num=0,
                )
            return gt, lo, hi

        def emit_scatter(cell_a, nslots, p2t):
            pr = cell_idx[0] % 2
            cell_idx[0] += 1
            if "scatter" in ABLATE:
                return
            for o in range(0, nslots, SCAT_MAX):
                e = min(nslots, o + SCAT_MAX)
                nc.gpsimd.dma_scatter_add(
                    acc[pr][0][:],
                    p2t[:].rearrange("p (r c) -> p r c", c=C)
                    [:, o // 128:e // 128, :],
                    dl_all[:, (cell_a + o) // 16:(cell_a + e) // 16],
                    e - o, e - o, C,
                    single_packet=False, queue_num=1 + cell_idx[0] % 3,
                    sbuf_tokens_per_rank=128,
                    parity_reg=0,
                    out_ap_other=acc[pr][1][:],
                )

        n_slab = [int(-(-int(plan.sreg_pad[sr]) // GSLAB))
                  for sr in range(N_SREG)]
        slab_seq = [(sr, si) for sr in range(N_SREG)
                    for si in range(n_slab[sr])]
        gts = {}
        LOOKAHEAD = OPT["gath_bufs"] - 1
        for j in range(min(LOOKAHEAD, len(slab_seq))):
            gts[j] = emit_gather(*slab_seq[j])

        open_p2t = {}

        def process_slab(j):
            sr, si = slab_seq[j]
            gt, lo, hi = gts.pop(j)
            sreg_lo = int(plan.sreg_off[sr])
            cell_a = [sreg_lo + int(plan.cell_off[sr, ki])
                      for ki in range(KG)]
            cell_b = [cell_a[ki] + int(plan.budgets[sr, ki])
                      for ki in range(KG)]
            for t0 in range(lo, hi, 512):
                t1 = min(hi, t0 + 512)
                ps1 = psum1.tile([128, 512], F32)
                for ki in range(KG):
                    a, b = max(cell_a[ki], t0), min(cell_b[ki], t1)
                    if a < b:
                        k = KG_LIST[ki]
                        nc.tensor.matmul(ps1[:, a - t0:b - t0],
                                         w_sb[:, k * C:(k + 1) * C],
                                         gt[:, 0, a - lo:b - lo],
                                         start=True, stop=True)
                p2 = p2sbp.tile([128, 512], BF16)
                nc.vector.tensor_copy(p2[:, :t1 - t0], ps1[:, :t1 - t0])
                psT = psumT.tile([128, 512], BF16)
                for jj in range((t1 - t0) // 128):
                    nc.tensor.transpose(psT[:, jj * 128:(jj + 1) * 128],
                                        p2[:, jj * 128:(jj + 1) * 128],
                                        identity[:])
                for ki in range(KG):
                    a, b = max(cell_a[ki], t0), min(cell_b[ki], t1)
                    if a >= b:
                        continue
                    if ki not in open_p2t:
                        open_p2t[ki] = p2tp.tile([128, plan.cmax], BF16,
                                                 name="p2t_cell")
                    nc.scalar.copy(
                        open_p2t[ki][:, a - cell_a[ki]:b - cell_a[ki]],
                        psT[:, a - t0:b - t0])
                    if b == cell_b[ki]:
                        emit_scatter(cell_a[ki], cell_b[ki] - cell_a[ki],
                                     open_p2t.pop(ki))

        def emit_tail(sr):
            """A += B merge + per-channel stats (PE AtA/ones matmuls)."""
            g0, g1 = SREG_GRP[sr]
            sl = slice(g0 * 128, g1 * 128)
            for pa in range(2):
                a_t = acc[0][pa][:, sl]
                nc.vector.tensor_tensor(out=a_t, in0=a_t,
                                        in1=acc[1][pa][:, sl],
                                        op=mybir.AluOpType.add)
            for g in range(g0, g1):
                for pa in range(2):
                    ch = acc[0][pa][:, g * 128:(g + 1) * 128]
                    first = (sr == 0 and g == g0 and pa == 0)
                    last = (sr == 1 and g == g1 - 1 and pa == 1)
                    nc.tensor.matmul(ps_stat[:, 0:128], ch, ch,
                                     start=first, stop=last)
                    nc.tensor.matmul(ps_stat[:, 128:129], ch, ones_col[:],
                                     start=first, stop=last)

        ps_stat = psumF.tile([128, 256], F32)

        sr_last = [n_slab[0] - 1, len(slab_seq) - 1]
        for j in range(len(slab_seq)):
            if j + LOOKAHEAD < len(slab_seq):
                gts[j + LOOKAHEAD] = emit_gather(*slab_seq[j + LOOKAHEAD])
            process_slab(j)
            if j == sr_last[0]:
                emit_tail(0)
        emit_tail(1)

        # ---- stats finalize + all-reduce ----
        stats_sb = constp.tile([128, 2], F32)
        diag_t = constp.tile([128, 128], F32)
        nc.vector.tensor_tensor(out=diag_t[:], in0=ps_stat[:, 0:128],
                                in1=identity[:], op=mybir.AluOpType.mult)
        nc.vector.tensor_reduce(out=stats_sb[:, 1:2], in_=diag_t[:],
                                axis=mybir.AxisListType.X,
                                op=mybir.AluOpType.add)
        nc.vector.tensor_copy(stats_sb[:, 0:1], ps_stat[:, 128:129])

        if n_cores > 1:
            stats_in = dramp.tile([128, 2], F32)
            stats_out = dramp.tile([128, 2], F32)
            nc.sync.dma_start(stats_in[:], stats_sb[:])
            nc.gpsimd.collective_compute(
                "AllReduce", mybir.AluOpType.add,
                replica_groups=[list(range(n_cores))],
                ins=[stats_in.opt()], outs=[stats_out.opt()],
            )
            stats2_sb = constp.tile([128, 2], F32)
            nc.sync.dma_start(stats2_sb[:], stats_out[:])
        else:
            stats2_sb = stats_sb

        mean_t = constp.tile([128, 1], F32)
        ex2_t = constp.tile([128, 1], F32)
        var_t = constp.tile([128, 1], F32)
        std_t = constp.tile([128, 1], F32)
        rstd_t = constp.tile([128, 1], F32)
        s_vec = constp.tile([128, 1], F32)
        t_vec = constp.tile([128, 1], F32)
        tmp = constp.tile([128, 1], F32)
        inv_n = 1.0 / N_TOTAL
        nc.vector.tensor_scalar_mul(mean_t[:], stats2_sb[:, 0:1], inv_n)
        nc.vector.tensor_scalar_mul(ex2_t[:], stats2_sb[:, 1:2], inv_n)
        nc.vector.tensor_tensor(out=tmp[:], in0=mean_t[:], in1=mean_t[:],
                                op=mybir.AluOpType.mult)
        nc.vector.tensor_tensor(out=var_t[:], in0=ex2_t[:], in1=tmp[:],
                                op=mybir.AluOpType.subtract)
        nc.vector.tensor_scalar_add(var_t[:], var_t[:], EPS)
        nc.scalar.activation(std_t[:], var_t[:],
                             mybir.ActivationFunctionType.Sqrt)
        nc.vector.reciprocal(rstd_t[:], std_t[:])
        nc.vector.tensor_tensor(out=s_vec[:], in0=rstd_t[:], in1=gamma_sb[:],
                                op=mybir.AluOpType.mult)
        nc.vector.tensor_tensor(out=tmp[:], in0=mean_t[:], in1=s_vec[:],
                                op=mybir.AluOpType.mult)
        nc.vector.tensor_tensor(out=t_vec[:], in0=beta_sb[:], in1=tmp[:],
                                op=mybir.AluOpType.subtract)

        # broadcast s/t across partitions into 1792-wide tiles
        s_bf = constp.tile([128, 1], BF16)
        t_bf = constp.tile([128, 1], BF16)
        nc.vector.tensor_copy(s_bf[:], s_vec[:])
        nc.vector.tensor_copy(t_bf[:], t_vec[:])
        psr = psumF.tile([1, 256], BF16)
        nc.tensor.transpose(psr[:, 0:128], s_bf[:], identity[:])
        nc.tensor.transpose(psr[:, 128:256], t_bf[:], identity[:])
        row_sb = constp.tile([1, 256], BF16)
        nc.vector.tensor_copy(row_sb[:], psr[:])
        psb = psumF.tile([128, 256], F32)
        nc.tensor.matmul(psb[:, 0:128], ones_row[:], row_sb[:, 0:128],
                         start=True, stop=True)
        nc.tensor.matmul(psb[:, 128:256], ones_row[:], row_sb[:, 128:256],
                         start=True, stop=True)
        s_wide = constp.tile([128, 7 * 128], F32)
        t_wide = constp.tile([128, 7 * 128], F32)
        for gsl in range(7):
            nc.vector.tensor_copy(s_wide[:, gsl * 128:(gsl + 1) * 128],
                                  psb[:, 0:128])
            nc.vector.tensor_copy(t_wide[:, gsl * 128:(gsl + 1) * 128],
                                  psb[:, 128:256])

        # ---- phase 2: BN + LeakyReLU + writeback [dst, c] rows ----
        out_v = out_ap.rearrange("(g two p) c -> two p g c", two=2, p=128)
        for pa in range(2):
            for s0 in range(0, N_GRP, 7):
                s1 = min(N_GRP, s0 + 7)
                n = (s1 - s0) * 128
                y = ph2p.tile([128, 7 * 128], F32, name="ph2y")
                nc.vector.tensor_tensor(
                    out=y[:, :n], in0=acc[0][pa][:, s0 * 128:s1 * 128],
                    in1=s_wide[:, :n], op=mybir.AluOpType.mult)
                nc.vector.tensor_tensor(
                    out=y[:, :n], in0=y[:, :n], in1=t_wide[:, :n],
                    op=mybir.AluOpType.add)
                nc.vector.scalar_tensor_tensor(y[:, :n], y[:, :n], LEAK,
                                               y[:, :n],
                                               op0=mybir.AluOpType.mult,
                                               op1=mybir.AluOpType.max)
                nc.sync.dma_start(
                    out_v[pa, :, s0:s1, :],
                    y[:, :n].rearrange("p (g c) -> p g c", c=128))


def _build_bass(plan, reps=1, n_cores=N_CORES):
    import concourse.bacc as bacc
    import concourse.mybir as mybir
    import concourse.tile as tile

    nc = bacc.Bacc("TRN2", target_bir_lowering=False, debug=False,
                   num_devices=n_cores, num_swdge_queues=OPT["queues"])
    F32 = mybir.dt.float32
    BF16 = mybir.dt.bfloat16
    I16 = mybir.dt.int16
    ins = {
        "table": nc.dram_tensor("table", [TABLE_ROWS, C], BF16,
                                kind="ExternalInput")[:, :],
        "table_t": nc.dram_tensor("table_t", [C, SHARD], BF16,
                                  kind="ExternalInput")[:, :],
        "idx": nc.dram_tensor("idx", [128, plan.p_total // 16], I16,
                              kind="ExternalInput")[:, :],
        "dl": nc.dram_tensor("dl", [128, plan.p_total // 16], I16,
                             kind="ExternalInput")[:, :],
        "w": nc.dram_tensor("w", [K, C, C], BF16, kind="ExternalInput")[:, :, :],
        "gamma": nc.dram_tensor("gamma", [C], F32, kind="ExternalInput")[:],
        "beta": nc.dram_tensor("beta", [C], F32, kind="ExternalInput")[:],
    }
    out = nc.dram_tensor("out", [SHARD, C], F32, kind="ExternalOutput")
    with tile.TileContext(nc) as tc:
        for _ in range(reps):
            emit_kernel(tc, out[:, :], ins, plan, n_cores=n_cores)
    nc.compile()
    return nc


def prepare(features, W, gamma, beta, nb):
    members_labels, ok = _partition_components(nb, nb.shape[0], N_CORES, SHARD)
    if not ok:
        return None, None, None
    cores, counts = _build_pairs(nb, members_labels)
    plan = _make_plan(counts)
    feats_bf = features.astype(ml_dtypes.bfloat16)
    w_bf = W.astype(ml_dtypes.bfloat16)
    core_maps = []
    for core in cores:
        m = _build_core_arrays(core, plan, feats_bf)
        m["w"] = w_bf
        m["gamma"] = gamma
        m["beta"] = beta
        core_maps.append(m)
    return plan, core_maps, cores


def _reference_fallback(features, w, b, gamma, beta, nb):
    feats = np.asarray(features, np.float32)
    wf = np.asarray(w, np.float32)
    out = np.broadcast_to(np.asarray(b, np.float32), feats.shape).copy()
    valid = nb >= 0
    idx = np.where(valid, nb, 0)
    for k in range(K):
        xk = feats[idx[:, k]] * valid[:, k:k + 1]
        out += xk @ wf[k]
    mean = out.mean(0)
    var = out.var(0)
    out = (out - mean) / np.sqrt(var + EPS) * np.asarray(gamma, np.float32) \
        + np.asarray(beta, np.float32)
    return np.where(out > 0, out, LEAK * out).astype(np.float32)


def kernel(features, W, b, gamma, beta, neighbor_idx):
    from concourse.bass_utils import run_bass_kernel_spmd

    features = np.asarray(features, np.float32)
    Wf = np.asarray(W, np.float32)
    gamma_f = np.asarray(gamma, np.float32)
    beta_f = np.asarray(beta, np.float32)
    nb = np.asarray(neighbor_idx, np.int32)
    assert features.shape == (N_TOTAL, C)

    plan, core_maps, cores = prepare(features, Wf, gamma_f, beta_f, nb)
    if plan is None:
        return _reference_fallback(features, Wf, b, gamma_f, beta_f, nb)

    nc = _build_bass(plan)
    res = run_bass_kernel_spmd(nc, core_maps, core_ids=list(range(N_CORES)))

    out_full = np.empty((N_TOTAL, C), np.float32)
    for c, core in enumerate(cores):
        out_full[core["mem"]] = res.results[c]["out"][core["rows"]]
    return out_full
